# revision 5
# baseline (speedup 1.0000x reference)
"""CorrelationLayer kernel for 8 TRN2 NeuronCores.

corr[b,0,i,j] = sum_c fmap1[b,c,i,j] * mean_{k,l} fmap2[b,c,k,l]

Sharding: data-parallel over B (B=8 -> one sample per core). Per core:
  fmap2 [256, 9216] streams through SBUF; per-channel sums reduced on DVE.
  fmap1 [256, 9216] loaded resident in SBUF.
  m2 [256,1] (scaled by 1/9216) used as stationary matmul weights; fmap1
  streams through the PE in [128, 512] tiles, PSUM-accumulated over the
  two 128-channel blocks -> out [1, 9216].
"""

import numpy as np

import concourse.bass as bass
import concourse.tile as tile
from concourse import bacc, mybir
from concourse.bass_utils import run_bass_kernel_spmd

B, C, H, W = 8, 256, 96, 96
HW = H * W            # 9216
P = 128
KB = C // P           # 2 channel blocks
F2_TILES = 4          # fmap2 stream tiles per channel block
F2T = HW // F2_TILES  # 2304
F1_CHUNKS = 6         # fmap1 DMA chunks per channel block
F1C = HW // F1_CHUNKS # 1536
N_T = 512             # matmul moving free dim (one PSUM bank)
N_TILES = HW // N_T   # 18
DT = mybir.dt.float32

_NC_CACHE = []


def _build():
    nc = bacc.Bacc("TRN2", debug=False)
    f1 = nc.dram_tensor("fmap1", [C, HW], DT, kind="ExternalInput").ap()
    f2 = nc.dram_tensor("fmap2", [C, HW], DT, kind="ExternalInput").ap()
    out = nc.dram_tensor("out", [1, HW], DT, kind="ExternalOutput").ap()

    with tile.TileContext(nc) as tc:
        with (
            tc.tile_pool(name="f2p", bufs=1) as f2p,
            tc.tile_pool(name="f1p", bufs=1) as f1p,
            tc.tile_pool(name="stat", bufs=1) as statp,
            tc.tile_pool(name="outp", bufs=1) as outp,
            tc.tile_pool(name="psp", bufs=8, space="PSUM") as psp,
        ):
            # All input tiles are resident (unique tags, no slot reuse) so
            # every input DMA has zero sync waits — HW DMA descriptors
            # support at most one wait condition.

            # --- fmap2: stream + per-tile channel-sum reduce ---
            parts = [
                statp.tile([P, F2_TILES], DT, name=f"part{kb}", tag=f"part{kb}")
                for kb in range(KB)
            ]
            for kb in range(KB):
                for t in range(F2_TILES):
                    f2t = f2p.tile(
                        [P, F2T], DT, name=f"f2_{kb}_{t}", tag=f"f2_{kb}_{t}"
                    )
                    nc.sync.dma_start(
                        out=f2t[:],
                        in_=f2[kb * P:(kb + 1) * P, t * F2T:(t + 1) * F2T],
                    )
                    nc.vector.reduce_sum(
                        parts[kb][:, t:t + 1], f2t[:], axis=mybir.AxisListType.X
                    )

            # --- fmap1: resident chunk loads ---
            f1c = {}
            for kb in range(KB):
                for j in range(F1_CHUNKS):
                    t_ = f1p.tile([P, F1C], DT, name=f"f1_{kb}_{j}", tag=f"f1_{kb}_{j}")
                    nc.sync.dma_start(
                        out=t_[:],
                        in_=f1[kb * P:(kb + 1) * P, j * F1C:(j + 1) * F1C],
                    )
                    f1c[(kb, j)] = t_

            # --- finalize m2 = mean over HW, per channel block ---
            m2 = []
            for kb in range(KB):
                raw = statp.tile([P, 1], DT, name=f"m2r{kb}", tag=f"m2r{kb}")
                nc.vector.reduce_sum(raw[:], parts[kb][:], axis=mybir.AxisListType.X)
                sc = statp.tile([P, 1], DT, name=f"m2s{kb}", tag=f"m2s{kb}")
                nc.scalar.mul(sc[:], raw[:], 1.0 / HW)
                m2.append(sc)

            # --- matvec: out[1, n] += m2_kb.T @ f1_kb[:, n] ---
            out_sb = outp.tile([1, HW], DT, name="out_sb", tag="out_sb")
            for nt in range(N_TILES):
                ps = psp.tile([1, N_T], DT, name="ps", tag="ps")
                j, local = divmod(nt * N_T, F1C)
                for kb in range(KB):
                    nc.tensor.matmul(
                        ps[:],
                        m2[kb][:],
                        f1c[(kb, j)][:, local:local + N_T],
                        start=(kb == 0),
                        stop=(kb == KB - 1),
                    )
                nc.scalar.copy(out_sb[:, nt * N_T:(nt + 1) * N_T], ps[:])
            nc.sync.dma_start(out=out[:], in_=out_sb[:])

    nc.compile()
    return nc


def kernel(fmap1: np.ndarray, fmap2: np.ndarray) -> np.ndarray:
    fmap1 = np.ascontiguousarray(np.asarray(fmap1, dtype=np.float32))
    fmap2 = np.ascontiguousarray(np.asarray(fmap2, dtype=np.float32))
    assert fmap1.shape == (B, C, H, W) and fmap2.shape == (B, C, H, W)

    if not _NC_CACHE:
        _NC_CACHE.append(_build())
    nc = _NC_CACHE[0]

    in_maps = [
        {
            "fmap1": fmap1[b].reshape(C, HW),
            "fmap2": fmap2[b].reshape(C, HW),
        }
        for b in range(B)
    ]
    res = run_bass_kernel_spmd(nc, in_maps, core_ids=list(range(B)))
    out = np.stack(
        [res.results[b]["out"].reshape(1, H, W) for b in range(B)], axis=0
    )
    return out.astype(np.float32)


# revision 7
# speedup vs baseline: 36.2307x; 36.2307x over previous
"""CorrelationLayer kernel for 8 TRN2 NeuronCores.

corr[b,0,i,j] = sum_c fmap1[b,c,i,j] * mean_{k,l} fmap2[b,c,k,l]

Sharding: data-parallel over B (B=8 -> one sample per core). Per core:
  fmap2 [256, 9216] streams through SBUF; per-channel sums reduced on DVE.
  fmap1 [256, 9216] loaded resident in SBUF.
  m2 [256,1] (scaled by 1/9216) used as stationary matmul weights; fmap1
  streams through the PE in [128, 512] tiles, PSUM-accumulated over the
  two 128-channel blocks -> out [1, 9216].
"""

import numpy as np

import concourse.bass as bass
import concourse.tile as tile
from concourse import bacc, mybir
from concourse.bass_utils import run_bass_kernel_spmd

B, C, H, W = 8, 256, 96, 96
HW = H * W            # 9216
P = 128
KB = C // P           # 2 channel blocks
F2_TILES = 4          # fmap2 stream tiles per channel block
F2T = HW // F2_TILES  # 2304
F1_CHUNKS = 6         # fmap1 DMA chunks per channel block
F1C = HW // F1_CHUNKS # 1536
N_T = 512             # matmul moving free dim (one PSUM bank)
N_TILES = HW // N_T   # 18
DT = mybir.dt.float32

_NC_CACHE = []


def _build(loop_reps=None):
    """loop_reps: if set, wrap the body in a hardware For-loop running it
    that many times — used only for device-time measurement (the per-call
    dispatch overhead through the PJRT tunnel dwarfs the kernel itself, so
    single-execution wall time is useless; the slope over reps isn't)."""
    nc = bacc.Bacc("TRN2", debug=False)
    f1 = nc.dram_tensor("fmap1", [C, HW], DT, kind="ExternalInput").ap()
    f2 = nc.dram_tensor("fmap2", [C, HW], DT, kind="ExternalInput").ap()
    out = nc.dram_tensor("out", [1, HW], DT, kind="ExternalOutput").ap()

    with tile.TileContext(nc) as tc:
        ctx_loop = tc.For_i(0, loop_reps, 1) if loop_reps else None
        if ctx_loop is not None:
            ctx_loop.__enter__()
        with (
            tc.tile_pool(name="f2p", bufs=1) as f2p,
            tc.tile_pool(name="f1p", bufs=1) as f1p,
            tc.tile_pool(name="stat", bufs=1) as statp,
            tc.tile_pool(name="outp", bufs=1) as outp,
            tc.tile_pool(name="psp", bufs=8, space="PSUM") as psp,
        ):
            # All input tiles are resident (unique tags, no slot reuse) so
            # every input DMA has zero sync waits — HW DMA descriptors
            # support at most one wait condition.

            # --- fmap2: stream + per-tile channel-sum reduce ---
            parts = [
                statp.tile([P, F2_TILES], DT, name=f"part{kb}", tag=f"part{kb}")
                for kb in range(KB)
            ]
            for kb in range(KB):
                for t in range(F2_TILES):
                    f2t = f2p.tile(
                        [P, F2T], DT, name=f"f2_{kb}_{t}", tag=f"f2_{kb}_{t}"
                    )
                    nc.sync.dma_start(
                        out=f2t[:],
                        in_=f2[kb * P:(kb + 1) * P, t * F2T:(t + 1) * F2T],
                    )
                    nc.vector.reduce_sum(
                        parts[kb][:, t:t + 1], f2t[:], axis=mybir.AxisListType.X
                    )

            # --- fmap1: resident chunk loads ---
            f1c = {}
            for kb in range(KB):
                for j in range(F1_CHUNKS):
                    t_ = f1p.tile([P, F1C], DT, name=f"f1_{kb}_{j}", tag=f"f1_{kb}_{j}")
                    nc.sync.dma_start(
                        out=t_[:],
                        in_=f1[kb * P:(kb + 1) * P, j * F1C:(j + 1) * F1C],
                    )
                    f1c[(kb, j)] = t_

            # --- finalize m2 = mean over HW, per channel block ---
            m2 = []
            for kb in range(KB):
                raw = statp.tile([P, 1], DT, name=f"m2r{kb}", tag=f"m2r{kb}")
                nc.vector.reduce_sum(raw[:], parts[kb][:], axis=mybir.AxisListType.X)
                sc = statp.tile([P, 1], DT, name=f"m2s{kb}", tag=f"m2s{kb}")
                nc.scalar.mul(sc[:], raw[:], 1.0 / HW)
                m2.append(sc)

            # --- matvec: out[1, n] += m2_kb.T @ f1_kb[:, n] ---
            out_sb = outp.tile([1, HW], DT, name="out_sb", tag="out_sb")
            for nt in range(N_TILES):
                ps = psp.tile([1, N_T], DT, name="ps", tag="ps")
                j, local = divmod(nt * N_T, F1C)
                for kb in range(KB):
                    nc.tensor.matmul(
                        ps[:],
                        m2[kb][:],
                        f1c[(kb, j)][:, local:local + N_T],
                        start=(kb == 0),
                        stop=(kb == KB - 1),
                    )
                nc.scalar.copy(out_sb[:, nt * N_T:(nt + 1) * N_T], ps[:])
            nc.sync.dma_start(out=out[:], in_=out_sb[:])
        if ctx_loop is not None:
            ctx_loop.__exit__(None, None, None)

    nc.compile()
    return nc


def kernel(fmap1: np.ndarray, fmap2: np.ndarray) -> np.ndarray:
    fmap1 = np.ascontiguousarray(np.asarray(fmap1, dtype=np.float32))
    fmap2 = np.ascontiguousarray(np.asarray(fmap2, dtype=np.float32))
    assert fmap1.shape == (B, C, H, W) and fmap2.shape == (B, C, H, W)

    if not _NC_CACHE:
        _NC_CACHE.append(_build())
    nc = _NC_CACHE[0]

    in_maps = [
        {
            "fmap1": fmap1[b].reshape(C, HW),
            "fmap2": fmap2[b].reshape(C, HW),
        }
        for b in range(B)
    ]
    res = run_bass_kernel_spmd(nc, in_maps, core_ids=list(range(B)))
    out = np.stack(
        [res.results[b]["out"].reshape(1, H, W) for b in range(B)], axis=0
    )
    return out.astype(np.float32)


# revision 11
# speedup vs baseline: 60.4755x; 1.6692x over previous
"""CorrelationLayer kernel for 8 TRN2 NeuronCores.

corr[b,0,i,j] = sum_c fmap1[b,c,i,j] * mean_{k,l} fmap2[b,c,k,l]

Sharding: data-parallel over B (B=8 -> one sample per core). Per core:
  fmap2 [256, 9216] streams through SBUF; per-channel sums reduced on DVE.
  fmap1 [256, 9216] loaded resident in SBUF.
  m2 [256,1] (scaled by 1/9216) used as stationary matmul weights; fmap1
  streams through the PE in [128, 512] tiles, PSUM-accumulated over the
  two 128-channel blocks -> out [1, 9216].
"""

import numpy as np

import concourse.bass as bass
import concourse.tile as tile
from concourse import bacc, mybir
from concourse.bass_utils import run_bass_kernel_spmd

B, C, H, W = 8, 256, 96, 96
HW = H * W            # 9216
P = 128
KB = C // P           # 2 channel blocks
F2_TILES = 4          # fmap2 stream tiles per channel block
F2T = HW // F2_TILES  # 2304
F1_CHUNKS = 6         # fmap1 DMA chunks per channel block
F1C = HW // F1_CHUNKS # 1536
N_T = 512             # matmul moving free dim (one PSUM bank)
N_TILES = HW // N_T   # 18
DT = mybir.dt.float32

_NC_CACHE = []


def _build(loop_reps=None, dma_only=False):
    """loop_reps: if set, wrap the body in a hardware For-loop running it
    that many times — used only for device-time measurement (the per-call
    dispatch overhead through the PJRT tunnel dwarfs the kernel itself, so
    single-execution wall time is useless; the slope over reps isn't).
    dma_only: emit just the input DMAs (device DMA-floor measurement)."""
    nc = bacc.Bacc("TRN2", debug=False)
    f1 = nc.dram_tensor("fmap1", [C, HW], DT, kind="ExternalInput").ap()
    f2 = nc.dram_tensor("fmap2", [C, HW], DT, kind="ExternalInput").ap()
    out = nc.dram_tensor("out", [1, HW], DT, kind="ExternalOutput").ap()

    with tile.TileContext(nc) as tc:
        ctx_loop = tc.For_i(0, loop_reps, 1) if loop_reps else None
        if ctx_loop is not None:
            ctx_loop.__enter__()
        with (
            tc.tile_pool(name="f2p", bufs=1) as f2p,
            tc.tile_pool(name="f1p", bufs=1) as f1p,
            tc.tile_pool(name="stat", bufs=1) as statp,
            tc.tile_pool(name="outp", bufs=1) as outp,
            tc.tile_pool(name="psp", bufs=8, space="PSUM") as psp,
        ):
            # All input tiles are resident (unique tags, no slot reuse) so
            # every input DMA has zero sync waits — HW DMA descriptors
            # support at most one wait condition.

            # --- fmap2: stream + per-tile channel-sum reduce ---
            parts = [
                statp.tile([P, F2_TILES], DT, name=f"part{kb}", tag=f"part{kb}")
                for kb in range(KB)
            ]
            for kb in range(KB):
                for t in range(F2_TILES):
                    f2t = f2p.tile(
                        [P, F2T], DT, name=f"f2_{kb}_{t}", tag=f"f2_{kb}_{t}"
                    )
                    nc.sync.dma_start(
                        out=f2t[:],
                        in_=f2[kb * P:(kb + 1) * P, t * F2T:(t + 1) * F2T],
                    )
                    if not dma_only:
                        nc.vector.reduce_sum(
                            parts[kb][:, t:t + 1], f2t[:], axis=mybir.AxisListType.X
                        )

            # --- fmap1: resident chunk loads, kb-interleaved so both
            # channel blocks of the same columns land back-to-back in the
            # HWDGE ring (nt groups complete as early as possible) ---
            f1c = {}
            for j in range(F1_CHUNKS):
                for kb in range(KB):
                    t_ = f1p.tile([P, F1C], DT, name=f"f1_{kb}_{j}", tag=f"f1_{kb}_{j}")
                    nc.sync.dma_start(
                        out=t_[:],
                        in_=f1[kb * P:(kb + 1) * P, j * F1C:(j + 1) * F1C],
                    )
                    f1c[(kb, j)] = t_

            if not dma_only:
                # --- m2 = per-channel sums of fmap2 (1/HW folded into the
                # psum->sbuf copy so matmuls wait only on the DVE reduce) ---
                m2 = []
                for kb in range(KB):
                    raw = statp.tile([P, 1], DT, name=f"m2r{kb}", tag=f"m2r{kb}")
                    nc.vector.reduce_sum(
                        raw[:], parts[kb][:], axis=mybir.AxisListType.X
                    )
                    m2.append(raw)

                # --- matvec chase: per column group j, both kb matmuls,
                # scaled copy to SBUF, flush on the ACT HWDGE ring ---
                out_sb = outp.tile([1, HW], DT, name="out_sb", tag="out_sb")
                npg = F1C // N_T  # N-tiles per chunk group
                for j in range(F1_CHUNKS):
                    for g in range(npg):
                        nt = j * npg + g
                        ps = psp.tile([1, N_T], DT, name="ps", tag="ps")
                        local = g * N_T
                        for kb in range(KB):
                            nc.tensor.matmul(
                                ps[:],
                                m2[kb][:],
                                f1c[(kb, j)][:, local:local + N_T],
                                start=(kb == 0),
                                stop=(kb == KB - 1),
                            )
                        nc.scalar.mul(
                            out_sb[:, nt * N_T:(nt + 1) * N_T], ps[:], 1.0 / HW
                        )
                    nc.scalar.dma_start(
                        out=out[:, j * F1C:(j + 1) * F1C],
                        in_=out_sb[:, j * F1C:(j + 1) * F1C],
                    )
        if ctx_loop is not None:
            ctx_loop.__exit__(None, None, None)

    nc.compile()
    return nc


def kernel(fmap1: np.ndarray, fmap2: np.ndarray) -> np.ndarray:
    fmap1 = np.ascontiguousarray(np.asarray(fmap1, dtype=np.float32))
    fmap2 = np.ascontiguousarray(np.asarray(fmap2, dtype=np.float32))
    assert fmap1.shape == (B, C, H, W) and fmap2.shape == (B, C, H, W)

    if not _NC_CACHE:
        _NC_CACHE.append(_build())
    nc = _NC_CACHE[0]

    in_maps = [
        {
            "fmap1": fmap1[b].reshape(C, HW),
            "fmap2": fmap2[b].reshape(C, HW),
        }
        for b in range(B)
    ]
    res = run_bass_kernel_spmd(nc, in_maps, core_ids=list(range(B)))
    out = np.stack(
        [res.results[b]["out"].reshape(1, H, W) for b in range(B)], axis=0
    )
    return out.astype(np.float32)
